# revision 23
# baseline (speedup 1.0000x reference)
"""Trainium2 Bass kernel for FK model (nn_FKModelWithProjection).

The reference's output algebraically reduces to the world-space joint
positions of a 24-joint SMPL kinematic chain:
    R_j = Rodrigues(poses[:, j])
    W_0 = R_0,  W_j = W_par(j) @ R_j
    p_0 = canon[0],  p_j = p_par(j) + W_par(j) @ (canon[j] - canon[par(j)])
    out[f, j] = p_j(f)

Device strategy (data parallel over frames, 8 cores):
  - rotations carried as quaternions (cheapest composition: 16 mul + 12 add)
  - all per-frame scalars live as [128, 98] column blocks (frame-major);
    per-joint quantities are adjacent column blocks so whole tree LEVELS
    are processed by single fat DVE instructions
  - quaternion Hamilton product done with 4 "XOR-permutation" access
    patterns (the Klein-group structure of quat index products lets each
    first-operand component hit all 4 second-operand components with one
    strided AP), then sign-folded combines
  - position update p += t + 2(qw*(qv x t) + qv x (qv x t)) with the
    constant bone vectors t broadcast from a tiny const tile (step-0 APs)
  - transcendentals (sqrt/sin/ln/exp) on the scalar (ACT) engine
"""

import numpy as np

import concourse.bass as bass
import concourse.tile as tile
from concourse import bacc, mybir
from concourse.bass_utils import run_bass_kernel_spmd

AF = mybir.ActivationFunctionType
OP = mybir.AluOpType

PARENTS = np.array(
    [0, 0, 0, 0, 1, 2, 3, 4, 5, 6, 7, 8, 9, 9, 9, 12, 13, 14, 16, 17, 18, 19, 20, 21],
    dtype=np.int64,
)
J = 24
NCORES = 8
P = 128          # SBUF partitions
C = 98           # frame columns per (joint, quantity) block
FC = P * C       # frames per core = 12544
FTOT = NCORES * FC  # padded total = 100352
F = 100000
S = J * C        # comp-plane stride in the big tiles = 2352
UW = 5 * C       # u-tile block stride (max level width = 5 joints... use 490)

# (child0, n_children, parent0, parent_broadcast)
# compose groups: non-leaf joints only (leaves {10,11,15,22,23} never parent)
COMPOSE_GROUPS = [
    (1, 3, 0, True),
    (4, 3, 1, False),
    (7, 3, 4, False),
    (12, 3, 9, True),
    (16, 2, 13, False),
    (18, 2, 16, False),
    (20, 2, 18, False),
]
# position-update groups: every joint 1..23, children contiguous per group
P_GROUPS = [
    (1, 3, 0, True),
    (4, 3, 1, False),
    (7, 3, 4, False),
    (10, 3, 7, False),
    (13, 2, 9, True),
    (15, 3, 12, False),
    (18, 2, 16, False),
    (20, 2, 18, False),
    (22, 2, 20, False),
]

# const-tile column map
CT_T1 = 0     # (tz | tx | ty) planes of 24
CT_T2 = 72    # (ty | tz | tx)
CT_T0 = 144   # (tx | ty | tz)
CT_S1 = 216   # signs of x1-products  (-1, +1, -1, +1)
CT_S2 = 220   # signs of y1-products  (-1, +1, +1, -1)
CT_S3 = 224   # signs of z1-products  (-1, -1, +1, +1)
CT_HALFPI = 228
CT_C0 = 229   # canon[0] x,y,z
CT_W = 232

DT = mybir.dt.float32
NPDT = np.float32


def _ap(t: bass.AP, off: int, dims) -> bass.AP:
    """Build a custom AP on tile t: partition dim + given free [step, count] dims."""
    full = t
    pd = list(full.ap)[0]
    return bass.AP(
        tensor=full.tensor,
        offset=full.offset + off,
        ap=[list(pd)] + [list(d) for d in dims],
    )


def build_nc():
    nc = bacc.Bacc()
    poses_d = nc.dram_tensor("poses", [P, 3 * S], DT, kind="ExternalInput")
    consts_d = nc.dram_tensor("consts", [P, CT_W], DT, kind="ExternalInput")
    out_d = nc.dram_tensor("out", [P, 3 * S], DT, kind="ExternalOutput")

    with tile.TileContext(nc) as tc:
        with tc.tile_pool(name="persist", bufs=1) as perpool:
            CT = perpool.tile([P, CT_W], DT, name="CT")
            # Q: comp blocks [w | x | y | z | x' | y'], each S wide
            Q = perpool.tile([P, 6 * S], DT, name="Q")
            # P positions: comp planes [px | py | pz]
            PT = perpool.tile([P, 3 * S], DT, name="PT")
            # u scratch: blocks [ux | uy | uz | ux' | uy']
            U = perpool.tile([P, 5 * UW], DT, name="U")

            nc.sync.dma_start(CT[:, :], consts_d[:, :])

            # ---- Phase 1: local quaternions, 24 joints in 2 halves so the
            # ---- early tree levels can start composing while the ACT chain
            # ---- still runs on the second half of the joints.
            with (
                tc.tile_pool(name="ph1po", bufs=1) as ph1po,
                tc.tile_pool(name="ph1", bufs=2) as ph1,
            ):
                PO = ph1po.tile([P, 3 * S], DT, name="PO")
                # small first chunk (joints 0-7) lets levels 1-2 compose while
                # the ACT chain still runs on joints 8-23
                CHUNKS = [(0, 8 * C), (8 * C, 16 * C)]
                for o, H in CHUNKS:
                    for pl in range(3):
                        nc.sync.dma_start(
                            PO[:, pl * S + o : pl * S + o + H],
                            poses_d[:, pl * S + o : pl * S + o + H],
                        )
                for h, (o, H) in enumerate(CHUNKS):
                    A = ph1.tile([P, H], DT, name=f"A{h}", tag="A")
                    B = ph1.tile([P, H], DT, name=f"B{h}", tag="B")
                    C2 = ph1.tile([P, H], DT, name=f"C2{h}", tag="C2")
                    TH = ph1.tile([P, H], DT, name=f"TH{h}", tag="TH")
                    SH = ph1.tile([P, H], DT, name=f"SH{h}", tag="SH")
                    RT = ph1.tile([P, H], DT, name=f"RT{h}", tag="RT")
                    M = ph1.tile([P, H], DT, name=f"M{h}", tag="M")
                    px = PO[:, o : o + H]
                    py = PO[:, S + o : S + o + H]
                    pz = PO[:, 2 * S + o : 2 * S + o + H]
                    nc.scalar.activation(A[:, :], px, AF.Square)
                    nc.scalar.activation(B[:, :], py, AF.Square)
                    nc.vector.tensor_tensor(C2[:, :], A[:, :], B[:, :], OP.add)
                    nc.scalar.activation(A[:, :], pz, AF.Square)
                    nc.vector.tensor_tensor(B[:, :], C2[:, :], A[:, :], OP.add)
                    nc.vector.tensor_scalar_max(C2[:, :], B[:, :], 1e-12)
                    nc.scalar.activation(TH[:, :], C2[:, :], AF.Sqrt)
                    nc.scalar.activation(
                        Q[:, o : o + H], TH[:, :], AF.Sin,
                        bias=CT[:, CT_HALFPI : CT_HALFPI + 1], scale=0.5,
                    )
                    nc.scalar.activation(SH[:, :], TH[:, :], AF.Sin, scale=0.5)
                    nc.scalar.activation(A[:, :], C2[:, :], AF.Ln)
                    nc.scalar.activation(RT[:, :], A[:, :], AF.Exp, scale=-0.5)
                    nc.vector.tensor_tensor(M[:, :], SH[:, :], RT[:, :], OP.mult)
                    nc.vector.tensor_tensor(
                        _ap(Q, S + o, [[S, 3], [1, H]]),
                        _ap(M, 0, [[0, 3], [1, H]]),
                        _ap(PO, o, [[S, 3], [1, H]]),
                        OP.mult,
                    )
                # dup x,y of root joint 0 into x',y'
                nc.vector.tensor_copy(
                    _ap(Q, 4 * S, [[S, 2], [1, C]]), _ap(Q, S, [[S, 2], [1, C]])
                )
                # seed P for joint 0 = canon[0]
                nc.vector.tensor_copy(
                    _ap(PT, 0, [[S, 3], [1, C]]), _ap(CT, CT_C0, [[1, 3], [0, C]])
                )

            # ---------------- Phase 2: tree levels --------------------------
            with tc.tile_pool(name="lvl", bufs=2) as tp:
                for gi, (c0, n, p0, bc) in enumerate(COMPOSE_GROUPS):
                    nC = n * C
                    co, po = c0 * C, p0 * C
                    pstep = 0 if bc else 1
                    T0t = tp.tile([P, 4 * nC], DT, name=f"cp0_{gi}", tag="cp0")
                    T1t = tp.tile([P, 4 * nC], DT, name=f"cp1_{gi}", tag="cp1")
                    T2t = tp.tile([P, 4 * nC], DT, name=f"cp2_{gi}", tag="cp2")
                    T3t = tp.tile([P, 4 * nC], DT, name=f"cp3_{gi}", tag="cp3")
                    def par_b(blk):
                        if bc:
                            return _ap(Q, blk * S + po, [[0, 4], [0, n], [1, C]])
                        return _ap(Q, blk * S + po, [[0, 4], [1, nC]])

                    child4 = _ap(Q, 0 * S + co, [[S, 4], [1, nC]])
                    perm1 = _ap(Q, 1 * S + co, [[2 * S, 2], [-S, 2], [1, nC]])
                    perm2 = _ap(Q, 2 * S + co, [[-2 * S, 2], [S, 2], [1, nC]])
                    perm3 = _ap(Q, 3 * S + co, [[-2 * S, 2], [-S, 2], [1, nC]])
                    nc.vector.tensor_tensor(T0t[:, :], par_b(0), child4, OP.mult)
                    nc.vector.tensor_tensor(T1t[:, :], par_b(1), perm1, OP.mult)
                    nc.vector.tensor_tensor(T2t[:, :], par_b(2), perm2, OP.mult)
                    nc.vector.tensor_tensor(T3t[:, :], par_b(3), perm3, OP.mult)
                    # combine with signs folded into paired add/sub stages
                    # (A/B tiles hold blocks in (w,y,x,z) order):
                    #   qw = (P0w-P1w) - (P2w+P3w)    qx = (P0x+P1x) + (P2x-P3x)
                    #   qy = (P0y-P1y) + (P2y+P3y)    qz = (P0z+P1z) - (P2z-P3z)
                    At = tp.tile([P, 4 * nC], DT, name=f"cA_{gi}", tag="cA")
                    Bt = tp.tile([P, 4 * nC], DT, name=f"cB_{gi}", tag="cB")
                    wy_in = lambda t: _ap(t, 0, [[2 * nC, 2], [1, nC]])
                    xz_in = lambda t: _ap(t, nC, [[2 * nC, 2], [1, nC]])
                    nc.vector.tensor_tensor(
                        _ap(At, 0, [[1, 2 * nC]]), wy_in(T0t), wy_in(T1t), OP.subtract
                    )
                    nc.vector.tensor_tensor(
                        _ap(At, 2 * nC, [[1, 2 * nC]]), xz_in(T0t), xz_in(T1t), OP.add
                    )
                    nc.vector.tensor_tensor(
                        _ap(Bt, 0, [[1, 2 * nC]]), wy_in(T2t), wy_in(T3t), OP.add
                    )
                    nc.vector.tensor_tensor(
                        _ap(Bt, 2 * nC, [[1, 2 * nC]]), xz_in(T2t), xz_in(T3t),
                        OP.subtract,
                    )
                    # F: (w,z) = A-B -> Q blocks 0,3 ; (y,x) = A+B -> Q blocks 2,1
                    nc.vector.tensor_tensor(
                        _ap(Q, co, [[3 * S, 2], [1, nC]]),
                        _ap(At, 0, [[3 * nC, 2], [1, nC]]),
                        _ap(Bt, 0, [[3 * nC, 2], [1, nC]]),
                        OP.subtract,
                    )
                    nc.vector.tensor_tensor(
                        _ap(Q, 2 * S + co, [[-S, 2], [1, nC]]),
                        _ap(At, nC, [[nC, 2], [1, nC]]),
                        _ap(Bt, nC, [[nC, 2], [1, nC]]),
                        OP.add,
                    )
                    # dup new world x,y into x',y'
                    nc.vector.tensor_copy(
                        _ap(Q, 4 * S + co, [[S, 2], [1, nC]]),
                        _ap(Q, 1 * S + co, [[S, 2], [1, nC]]),
                    )

                for gi, (c0, n, p0, bc) in enumerate(P_GROUPS):
                    nC = n * C
                    co, po = c0 * C, p0 * C

                    def prot(blk0):
                        if bc:
                            return _ap(Q, blk0 * S + po, [[S, 3], [0, n], [1, C]])
                        return _ap(Q, blk0 * S + po, [[S, 3], [1, nC]])

                    w1b = (
                        _ap(Q, po, [[0, 3], [0, n], [1, C]])
                        if bc
                        else _ap(Q, po, [[0, 3], [1, nC]])
                    )
                    rot1 = prot(2)  # (qy, qz, qx')
                    rot2 = prot(3)  # (qz, qx', qy')
                    # T1/T2 host-prescaled by 2 => u,v,h all carry the final 2x
                    T1b = _ap(CT, CT_T1 + c0, [[24, 3], [1, n], [0, C]])
                    T2b = _ap(CT, CT_T2 + c0, [[24, 3], [1, n], [0, C]])
                    T0b = _ap(CT, CT_T0 + c0, [[24, 3], [1, n], [0, C]])
                    Ma = tp.tile([P, 3 * nC], DT, name=f"pa_{gi}", tag="pa")
                    Mb = tp.tile([P, 3 * nC], DT, name=f"pb_{gi}", tag="pb")
                    PRE = tp.tile([P, 3 * nC], DT, name=f"pre_{gi}", tag="pre")
                    ppar = (
                        _ap(PT, po, [[S, 3], [0, n], [1, C]])
                        if bc
                        else _ap(PT, po, [[S, 3], [1, nC]])
                    )
                    # hoistable off the u/v critical chain
                    nc.vector.tensor_tensor(PRE[:, :], ppar, T0b, OP.add)
                    nc.vector.tensor_tensor(Ma[:, :], rot1, T1b, OP.mult)
                    nc.vector.tensor_tensor(Mb[:, :], rot2, T2b, OP.mult)
                    uout = _ap(U, 0, [[UW, 3], [1, nC]])
                    nc.vector.tensor_tensor(uout, Ma[:, :], Mb[:, :], OP.subtract)
                    nc.vector.tensor_copy(
                        _ap(U, 3 * UW, [[UW, 2], [1, nC]]),
                        _ap(U, 0, [[UW, 2], [1, nC]]),
                    )
                    urot1 = _ap(U, 1 * UW, [[UW, 3], [1, nC]])  # (uy, uz, ux')
                    urot2 = _ap(U, 2 * UW, [[UW, 3], [1, nC]])  # (uz, ux', uy')
                    nc.vector.tensor_tensor(Ma[:, :], rot1, urot2, OP.mult)
                    nc.vector.tensor_tensor(Mb[:, :], rot2, urot1, OP.mult)
                    nc.vector.tensor_tensor(Ma[:, :], Ma[:, :], Mb[:, :], OP.subtract)
                    u3 = _ap(U, 0, [[UW, 3], [1, nC]])
                    nc.vector.tensor_tensor(Mb[:, :], w1b, u3, OP.mult)
                    nc.vector.tensor_tensor(Ma[:, :], Mb[:, :], Ma[:, :], OP.add)
                    # p_child = g + (p_par + t)   (g pre-scaled via consts)
                    nc.vector.tensor_tensor(
                        _ap(PT, co, [[S, 3], [1, nC]]), Ma[:, :], PRE[:, :], OP.add
                    )
                    # stream out this group's columns while later levels run
                    nc.sync.dma_start(
                        _ap(out_d[:, :], co, [[S, 3], [1, nC]]),
                        _ap(PT, co, [[S, 3], [1, nC]]),
                    )

            # joint-0 columns
            nc.sync.dma_start(
                _ap(out_d[:, :], 0, [[S, 3], [1, C]]), _ap(PT, 0, [[S, 3], [1, C]])
            )
    nc.compile()
    return nc


def _host_consts(canon: np.ndarray) -> np.ndarray:
    t = canon - canon[PARENTS]  # (24, 3)
    t2 = 2.0 * t  # fold the final "2x" of the rotation formula into u
    row = np.zeros(CT_W, dtype=NPDT)
    row[CT_T1 : CT_T1 + 24] = t2[:, 2]
    row[CT_T1 + 24 : CT_T1 + 48] = t2[:, 0]
    row[CT_T1 + 48 : CT_T1 + 72] = t2[:, 1]
    row[CT_T2 : CT_T2 + 24] = t2[:, 1]
    row[CT_T2 + 24 : CT_T2 + 48] = t2[:, 2]
    row[CT_T2 + 48 : CT_T2 + 72] = t2[:, 0]
    row[CT_T0 : CT_T0 + 24] = t[:, 0]
    row[CT_T0 + 24 : CT_T0 + 48] = t[:, 1]
    row[CT_T0 + 48 : CT_T0 + 72] = t[:, 2]
    row[CT_S1 : CT_S1 + 4] = [-1.0, 1.0, -1.0, 1.0]
    row[CT_S2 : CT_S2 + 4] = [-1.0, 1.0, 1.0, -1.0]
    row[CT_S3 : CT_S3 + 4] = [-1.0, -1.0, 1.0, 1.0]
    row[CT_HALFPI] = np.pi / 2
    row[CT_C0 : CT_C0 + 3] = canon[0]
    return np.broadcast_to(row, (P, CT_W)).copy()


def _shard_poses(poses: np.ndarray) -> np.ndarray:
    pad = np.broadcast_to(poses[-1:], (FTOT - F, J, 3))
    pp = np.concatenate([poses.astype(NPDT), pad.astype(NPDT)], axis=0)
    x = pp.reshape(NCORES, P, C, J, 3).transpose(0, 1, 4, 3, 2)  # core,p,comp,j,c
    return np.ascontiguousarray(x).reshape(NCORES, P, 3 * S)


def _unshard_out(outs: np.ndarray) -> np.ndarray:
    y = outs.reshape(NCORES, P, 3, J, C).transpose(0, 1, 4, 3, 2)  # core,p,c,j,comp
    return np.ascontiguousarray(y).reshape(FTOT, J, 3)[:F]


_NC_CACHE = {}


def kernel(poses, canon_kpts, parents):
    poses = np.asarray(poses)
    canon = np.asarray(canon_kpts, dtype=NPDT)
    if "nc" not in _NC_CACHE:
        _NC_CACHE["nc"] = build_nc()
    nc = _NC_CACHE["nc"]
    shards = _shard_poses(poses)
    consts = _host_consts(canon)
    in_maps = [{"poses": shards[i], "consts": consts} for i in range(NCORES)]
    res = run_bass_kernel_spmd(nc, in_maps, list(range(NCORES))).results
    outs = np.stack([r["out"] for r in res])
    return _unshard_out(outs).astype(np.float32)


# revision 32
# speedup vs baseline: 1.0452x; 1.0452x over previous
"""Trainium2 Bass kernel for FK model (nn_FKModelWithProjection).

The reference's output algebraically reduces to the world-space joint
positions of a 24-joint SMPL kinematic chain:
    R_j = Rodrigues(poses[:, j])
    W_0 = R_0,  W_j = W_par(j) @ R_j
    p_0 = canon[0],  p_j = p_par(j) + W_par(j) @ (canon[j] - canon[par(j)])
    out[f, j] = p_j(f)

Device strategy (data parallel over frames, 8 cores):
  - rotations carried as quaternions (cheapest composition: 16 mul + 12 add)
  - all per-frame scalars live as [128, 98] column blocks (frame-major);
    per-joint quantities are adjacent column blocks so whole tree LEVELS
    are processed by single fat DVE instructions
  - quaternion Hamilton product done with 4 "XOR-permutation" access
    patterns (the Klein-group structure of quat index products lets each
    first-operand component hit all 4 second-operand components with one
    strided AP), then sign-folded combines
  - position update p += t + 2(qw*(qv x t) + qv x (qv x t)) with the
    constant bone vectors t broadcast from a tiny const tile (step-0 APs)
  - transcendentals (sqrt/sin/ln/exp) on the scalar (ACT) engine
"""

import numpy as np

import concourse.bass as bass
import concourse.tile as tile
from concourse import bacc, mybir
from concourse.bass_utils import run_bass_kernel_spmd

AF = mybir.ActivationFunctionType
OP = mybir.AluOpType

PARENTS = np.array(
    [0, 0, 0, 0, 1, 2, 3, 4, 5, 6, 7, 8, 9, 9, 9, 12, 13, 14, 16, 17, 18, 19, 20, 21],
    dtype=np.int64,
)
J = 24
NCORES = 8
P = 128          # SBUF partitions
C = 98           # frame columns per (joint, quantity) block
FC = P * C       # frames per core = 12544
FTOT = NCORES * FC  # padded total = 100352
F = 100000
S = J * C        # comp-plane stride in the big tiles = 2352
UW = 5 * C       # u-tile block stride (max level width = 5 joints... use 490)

# (child0, n_children, parent0, parent_broadcast)
# compose groups: non-leaf joints only (leaves {10,11,15,22,23} never parent)
COMPOSE_GROUPS = [
    (1, 3, 0, True),
    (4, 3, 1, False),
    (7, 3, 4, False),
    (12, 3, 9, True),
    (16, 2, 13, False),
    (18, 2, 16, False),
    (20, 2, 18, False),
]
# position-update groups: every joint 1..23, children contiguous per group
P_GROUPS = [
    (1, 3, 0, True),
    (4, 3, 1, False),
    (7, 3, 4, False),
    (10, 3, 7, False),
    (13, 2, 9, True),
    (15, 3, 12, False),
    (18, 2, 16, False),
    (20, 2, 18, False),
    (22, 2, 20, False),
]

# const-tile column map
CT_T1 = 0     # (tz | tx | ty) planes of 24
CT_T2 = 72    # (ty | tz | tx)
CT_T0 = 144   # (tx | ty | tz)
CT_S1 = 216   # signs of x1-products  (-1, +1, -1, +1)
CT_S2 = 220   # signs of y1-products  (-1, +1, +1, -1)
CT_S3 = 224   # signs of z1-products  (-1, -1, +1, +1)
CT_HALFPI = 228
CT_C0 = 229   # canon[0] x,y,z
CT_W = 232

DT = mybir.dt.float32
NPDT = np.float32


def _ap(t: bass.AP, off: int, dims) -> bass.AP:
    """Build a custom AP on tile t: partition dim + given free [step, count] dims."""
    full = t
    pd = list(full.ap)[0]
    return bass.AP(
        tensor=full.tensor,
        offset=full.offset + off,
        ap=[list(pd)] + [list(d) for d in dims],
    )


def build_nc():
    nc = bacc.Bacc()
    poses_d = nc.dram_tensor("poses", [P, 3 * S], DT, kind="ExternalInput")
    consts_d = nc.dram_tensor("consts", [P, CT_W], DT, kind="ExternalInput")
    out_d = nc.dram_tensor("out", [P, 3 * S], DT, kind="ExternalOutput")

    with tile.TileContext(nc) as tc:
        with tc.tile_pool(name="persist", bufs=1) as perpool:
            CT = perpool.tile([P, CT_W], DT, name="CT")
            # Q: comp blocks [w | x | y | z | x' | y'], each S wide
            Q = perpool.tile([P, 6 * S], DT, name="Q")
            # P positions: comp planes [px | py | pz]
            PT = perpool.tile([P, 3 * S], DT, name="PT")
            # u scratch: blocks [ux | uy | uz | ux' | uy']
            U = perpool.tile([P, 5 * UW], DT, name="U")

            nc.sync.dma_start(CT[:, :], consts_d[:, :])

            # ---- Phase 1: local quaternions, 24 joints in 2 halves so the
            # ---- early tree levels can start composing while the ACT chain
            # ---- still runs on the second half of the joints.
            with (
                tc.tile_pool(name="ph1po", bufs=1) as ph1po,
                tc.tile_pool(name="ph1", bufs=2) as ph1,
            ):
                PO = ph1po.tile([P, 3 * S], DT, name="PO")
                # quats are only consumed for the 19 non-leaf joints; the
                # non-leaf set {0-9, 12-14, 16-21} is 3 contiguous runs, so
                # skip leaf columns entirely. The first chunk (joints 0-9)
                # covers levels 1-3 so composing starts while the ACT chain
                # still runs on the later joints.
                CHUNKS = [(0, 10 * C), (12 * C, 3 * C), (16 * C, 6 * C)]
                for o, H in CHUNKS:
                    for pl in range(3):
                        nc.sync.dma_start(
                            PO[:, pl * S + o : pl * S + o + H],
                            poses_d[:, pl * S + o : pl * S + o + H],
                        )
                for h, (o, H) in enumerate(CHUNKS):
                    A = ph1.tile([P, H], DT, name=f"A{h}", tag="A")
                    B = ph1.tile([P, H], DT, name=f"B{h}", tag="B")
                    C2 = ph1.tile([P, H], DT, name=f"C2{h}", tag="C2")
                    TH = ph1.tile([P, H], DT, name=f"TH{h}", tag="TH")
                    SH = ph1.tile([P, H], DT, name=f"SH{h}", tag="SH")
                    RT = ph1.tile([P, H], DT, name=f"RT{h}", tag="RT")
                    M = ph1.tile([P, H], DT, name=f"M{h}", tag="M")
                    px = PO[:, o : o + H]
                    py = PO[:, S + o : S + o + H]
                    pz = PO[:, 2 * S + o : 2 * S + o + H]
                    # squares on DVE (idle during startup) so the ACT chain is
                    # a single sqrt/sin/ln/exp block per chunk, no ping-pong
                    nc.vector.tensor_tensor(A[:, :], px, px, OP.mult)
                    nc.vector.tensor_tensor(B[:, :], py, py, OP.mult)
                    nc.vector.tensor_tensor(A[:, :], A[:, :], B[:, :], OP.add)
                    nc.vector.tensor_tensor(B[:, :], pz, pz, OP.mult)
                    # guarded theta^2: max(zz,eps)+(xx+yy) == max(theta2,eps)
                    # up to <=1e-12 absolute, fused in one STT
                    nc.vector.scalar_tensor_tensor(
                        C2[:, :], B[:, :], 1e-12, A[:, :], OP.max, OP.add
                    )
                    nc.scalar.activation(TH[:, :], C2[:, :], AF.Sqrt)
                    nc.scalar.activation(
                        Q[:, o : o + H], TH[:, :], AF.Sin,
                        bias=CT[:, CT_HALFPI : CT_HALFPI + 1], scale=0.5,
                    )
                    nc.scalar.activation(SH[:, :], TH[:, :], AF.Sin, scale=0.5)
                    nc.scalar.activation(B[:, :], C2[:, :], AF.Ln)
                    nc.scalar.activation(RT[:, :], B[:, :], AF.Exp, scale=-0.5)
                    nc.vector.tensor_tensor(M[:, :], SH[:, :], RT[:, :], OP.mult)
                    nc.vector.tensor_tensor(
                        _ap(Q, S + o, [[S, 3], [1, H]]),
                        _ap(M, 0, [[0, 3], [1, H]]),
                        _ap(PO, o, [[S, 3], [1, H]]),
                        OP.mult,
                    )
                # dup x,y of root joint 0 into x',y'
                nc.vector.tensor_copy(
                    _ap(Q, 4 * S, [[S, 2], [1, C]]), _ap(Q, S, [[S, 2], [1, C]])
                )
                # seed P for joint 0 = canon[0]
                nc.vector.tensor_copy(
                    _ap(PT, 0, [[S, 3], [1, C]]), _ap(CT, CT_C0, [[1, 3], [0, C]])
                )

            # ---------------- Phase 2: tree levels --------------------------
            with tc.tile_pool(name="lvl", bufs=3) as tp:
                def compose_group(gi, c0, n, p0, bc):
                    nC = n * C
                    co, po = c0 * C, p0 * C
                    pstep = 0 if bc else 1
                    T0t = tp.tile([P, 4 * nC], DT, name=f"cp0_{gi}", tag="cp0")
                    T1t = tp.tile([P, 4 * nC], DT, name=f"cp1_{gi}", tag="cp1")
                    T2t = tp.tile([P, 4 * nC], DT, name=f"cp2_{gi}", tag="cp2")
                    T3t = tp.tile([P, 4 * nC], DT, name=f"cp3_{gi}", tag="cp3")
                    def par_b(blk):
                        if bc:
                            return _ap(Q, blk * S + po, [[0, 4], [0, n], [1, C]])
                        return _ap(Q, blk * S + po, [[0, 4], [1, nC]])

                    child4 = _ap(Q, 0 * S + co, [[S, 4], [1, nC]])
                    perm1 = _ap(Q, 1 * S + co, [[2 * S, 2], [-S, 2], [1, nC]])
                    perm2 = _ap(Q, 2 * S + co, [[-2 * S, 2], [S, 2], [1, nC]])
                    perm3 = _ap(Q, 3 * S + co, [[-2 * S, 2], [-S, 2], [1, nC]])
                    nc.vector.tensor_tensor(T0t[:, :], par_b(0), child4, OP.mult)
                    nc.vector.tensor_tensor(T1t[:, :], par_b(1), perm1, OP.mult)
                    nc.vector.tensor_tensor(T2t[:, :], par_b(2), perm2, OP.mult)
                    nc.vector.tensor_tensor(T3t[:, :], par_b(3), perm3, OP.mult)
                    # combine with signs folded into paired add/sub stages
                    # (A/B tiles hold blocks in (w,y,x,z) order):
                    #   qw = (P0w-P1w) - (P2w+P3w)    qx = (P0x+P1x) + (P2x-P3x)
                    #   qy = (P0y-P1y) + (P2y+P3y)    qz = (P0z+P1z) - (P2z-P3z)
                    At = tp.tile([P, 4 * nC], DT, name=f"cA_{gi}", tag="cA")
                    Bt = tp.tile([P, 4 * nC], DT, name=f"cB_{gi}", tag="cB")
                    wy_in = lambda t: _ap(t, 0, [[2 * nC, 2], [1, nC]])
                    xz_in = lambda t: _ap(t, nC, [[2 * nC, 2], [1, nC]])
                    nc.vector.tensor_tensor(
                        _ap(At, 0, [[1, 2 * nC]]), wy_in(T0t), wy_in(T1t), OP.subtract
                    )
                    nc.vector.tensor_tensor(
                        _ap(At, 2 * nC, [[1, 2 * nC]]), xz_in(T0t), xz_in(T1t), OP.add
                    )
                    nc.vector.tensor_tensor(
                        _ap(Bt, 0, [[1, 2 * nC]]), wy_in(T2t), wy_in(T3t), OP.add
                    )
                    nc.vector.tensor_tensor(
                        _ap(Bt, 2 * nC, [[1, 2 * nC]]), xz_in(T2t), xz_in(T3t),
                        OP.subtract,
                    )
                    # F: (w,z) = A-B -> Q blocks 0,3 ; (y,x) = A+B -> Q blocks 2,1
                    nc.vector.tensor_tensor(
                        _ap(Q, co, [[3 * S, 2], [1, nC]]),
                        _ap(At, 0, [[3 * nC, 2], [1, nC]]),
                        _ap(Bt, 0, [[3 * nC, 2], [1, nC]]),
                        OP.subtract,
                    )
                    nc.vector.tensor_tensor(
                        _ap(Q, 2 * S + co, [[-S, 2], [1, nC]]),
                        _ap(At, nC, [[nC, 2], [1, nC]]),
                        _ap(Bt, nC, [[nC, 2], [1, nC]]),
                        OP.add,
                    )
                    # dup new world x,y into x',y'
                    nc.vector.tensor_copy(
                        _ap(Q, 4 * S + co, [[S, 2], [1, nC]]),
                        _ap(Q, 1 * S + co, [[S, 2], [1, nC]]),
                    )

                def p_group(gi, c0, n, p0, bc):
                    nC = n * C
                    co, po = c0 * C, p0 * C

                    def prot(blk0):
                        if bc:
                            return _ap(Q, blk0 * S + po, [[S, 3], [0, n], [1, C]])
                        return _ap(Q, blk0 * S + po, [[S, 3], [1, nC]])

                    w1b = (
                        _ap(Q, po, [[0, 3], [0, n], [1, C]])
                        if bc
                        else _ap(Q, po, [[0, 3], [1, nC]])
                    )
                    rot1 = prot(2)  # (qy, qz, qx')
                    rot2 = prot(3)  # (qz, qx', qy')
                    # T1/T2 host-prescaled by 2 => u,v,h all carry the final 2x
                    T1b = _ap(CT, CT_T1 + c0, [[24, 3], [1, n], [0, C]])
                    T2b = _ap(CT, CT_T2 + c0, [[24, 3], [1, n], [0, C]])
                    T0b = _ap(CT, CT_T0 + c0, [[24, 3], [1, n], [0, C]])
                    Ma = tp.tile([P, 3 * nC], DT, name=f"pa_{gi}", tag="pa")
                    Mb = tp.tile([P, 3 * nC], DT, name=f"pb_{gi}", tag="pb")
                    PRE = tp.tile([P, 3 * nC], DT, name=f"pre_{gi}", tag="pre")
                    ppar = (
                        _ap(PT, po, [[S, 3], [0, n], [1, C]])
                        if bc
                        else _ap(PT, po, [[S, 3], [1, nC]])
                    )
                    # hoistable off the u/v critical chain
                    nc.vector.tensor_tensor(PRE[:, :], ppar, T0b, OP.add)
                    nc.vector.tensor_tensor(Ma[:, :], rot1, T1b, OP.mult)
                    nc.vector.tensor_tensor(Mb[:, :], rot2, T2b, OP.mult)
                    uout = _ap(U, 0, [[UW, 3], [1, nC]])
                    nc.vector.tensor_tensor(uout, Ma[:, :], Mb[:, :], OP.subtract)
                    nc.vector.tensor_copy(
                        _ap(U, 3 * UW, [[UW, 2], [1, nC]]),
                        _ap(U, 0, [[UW, 2], [1, nC]]),
                    )
                    urot1 = _ap(U, 1 * UW, [[UW, 3], [1, nC]])  # (uy, uz, ux')
                    urot2 = _ap(U, 2 * UW, [[UW, 3], [1, nC]])  # (uz, ux', uy')
                    nc.vector.tensor_tensor(Ma[:, :], rot1, urot2, OP.mult)
                    nc.vector.tensor_tensor(Mb[:, :], rot2, urot1, OP.mult)
                    nc.vector.tensor_tensor(Ma[:, :], Ma[:, :], Mb[:, :], OP.subtract)
                    u3 = _ap(U, 0, [[UW, 3], [1, nC]])
                    nc.vector.tensor_tensor(Mb[:, :], w1b, u3, OP.mult)
                    nc.vector.tensor_tensor(Ma[:, :], Mb[:, :], Ma[:, :], OP.add)
                    # p_child = g + (p_par + t)   (g pre-scaled via consts)
                    nc.vector.tensor_tensor(
                        _ap(PT, co, [[S, 3], [1, nC]]), Ma[:, :], PRE[:, :], OP.add
                    )
                    # stream out this group's columns while later levels run
                    nc.sync.dma_start(
                        _ap(out_d[:, :], co, [[S, 3], [1, nC]]),
                        _ap(PT, co, [[S, 3], [1, nC]]),
                    )

                # Emission order interleaves p-updates between compose levels
                # so the scheduler has DVE work to fill compose-chain waits:
                # H_k only needs level k-1 quats (already composed) + P chain.
                ORDER = [
                    ("p", 0), ("c", 0),          # H1 (needs only phase-1 j0) ; G1
                    ("p", 1), ("c", 1),          # H2 ; G2
                    ("p", 2), ("c", 2),          # H3 ; G3
                    ("p", 3), ("p", 4), ("c", 3),  # H4, H4b ; G4
                    ("p", 5), ("c", 4),          # H5 ; G5
                    ("p", 6), ("c", 5),          # H6 ; G6
                    ("p", 7), ("c", 6),          # H7 ; G7
                    ("p", 8),                    # H8
                ]
                for kind, gi in ORDER:
                    if kind == "c":
                        compose_group(gi, *COMPOSE_GROUPS[gi])
                    else:
                        p_group(gi, *P_GROUPS[gi])

            # joint-0 columns
            nc.sync.dma_start(
                _ap(out_d[:, :], 0, [[S, 3], [1, C]]), _ap(PT, 0, [[S, 3], [1, C]])
            )
    nc.compile()
    return nc


def _host_consts(canon: np.ndarray, parents: np.ndarray = None) -> np.ndarray:
    par = PARENTS if parents is None else np.asarray(parents).astype(np.int64)
    t = canon - canon[par]  # (24, 3)
    t2 = 2.0 * t  # fold the final "2x" of the rotation formula into u
    row = np.zeros(CT_W, dtype=NPDT)
    row[CT_T1 : CT_T1 + 24] = t2[:, 2]
    row[CT_T1 + 24 : CT_T1 + 48] = t2[:, 0]
    row[CT_T1 + 48 : CT_T1 + 72] = t2[:, 1]
    row[CT_T2 : CT_T2 + 24] = t2[:, 1]
    row[CT_T2 + 24 : CT_T2 + 48] = t2[:, 2]
    row[CT_T2 + 48 : CT_T2 + 72] = t2[:, 0]
    row[CT_T0 : CT_T0 + 24] = t[:, 0]
    row[CT_T0 + 24 : CT_T0 + 48] = t[:, 1]
    row[CT_T0 + 48 : CT_T0 + 72] = t[:, 2]
    row[CT_S1 : CT_S1 + 4] = [-1.0, 1.0, -1.0, 1.0]
    row[CT_S2 : CT_S2 + 4] = [-1.0, 1.0, 1.0, -1.0]
    row[CT_S3 : CT_S3 + 4] = [-1.0, -1.0, 1.0, 1.0]
    row[CT_HALFPI] = np.pi / 2
    row[CT_C0 : CT_C0 + 3] = canon[0]
    return np.broadcast_to(row, (P, CT_W)).copy()


def _shard_poses(poses: np.ndarray) -> np.ndarray:
    pad = np.broadcast_to(poses[-1:], (FTOT - F, J, 3))
    pp = np.concatenate([poses.astype(NPDT), pad.astype(NPDT)], axis=0)
    x = pp.reshape(NCORES, P, C, J, 3).transpose(0, 1, 4, 3, 2)  # core,p,comp,j,c
    return np.ascontiguousarray(x).reshape(NCORES, P, 3 * S)


def _unshard_out(outs: np.ndarray) -> np.ndarray:
    y = outs.reshape(NCORES, P, 3, J, C).transpose(0, 1, 4, 3, 2)  # core,p,c,j,comp
    return np.ascontiguousarray(y).reshape(FTOT, J, 3)[:F]


_NC_CACHE = {}


def kernel(poses, canon_kpts, parents):
    poses = np.asarray(poses)
    canon = np.asarray(canon_kpts, dtype=NPDT)
    if "nc" not in _NC_CACHE:
        _NC_CACHE["nc"] = build_nc()
    nc = _NC_CACHE["nc"]
    shards = _shard_poses(poses)
    consts = _host_consts(canon, parents)
    in_maps = [{"poses": shards[i], "consts": consts} for i in range(NCORES)]
    res = run_bass_kernel_spmd(nc, in_maps, list(range(NCORES))).results
    outs = np.stack([r["out"] for r in res])
    return _unshard_out(outs).astype(np.float32)


# revision 34
# speedup vs baseline: 1.0724x; 1.0260x over previous
"""Trainium2 Bass kernel for FK model (nn_FKModelWithProjection).

The reference's output algebraically reduces to the world-space joint
positions of a 24-joint SMPL kinematic chain:
    R_j = Rodrigues(poses[:, j])
    W_0 = R_0,  W_j = W_par(j) @ R_j
    p_0 = canon[0],  p_j = p_par(j) + W_par(j) @ (canon[j] - canon[par(j)])
    out[f, j] = p_j(f)

Device strategy (data parallel over frames, 8 cores):
  - rotations carried as quaternions (cheapest composition: 16 mul + 12 add)
  - all per-frame scalars live as [128, 98] column blocks (frame-major);
    per-joint quantities are adjacent column blocks so whole tree LEVELS
    are processed by single fat DVE instructions
  - quaternion Hamilton product done with 4 "XOR-permutation" access
    patterns (the Klein-group structure of quat index products lets each
    first-operand component hit all 4 second-operand components with one
    strided AP), then sign-folded combines
  - position update p += t + 2(qw*(qv x t) + qv x (qv x t)) with the
    constant bone vectors t broadcast from a tiny const tile (step-0 APs)
  - transcendentals (sqrt/sin/ln/exp) on the scalar (ACT) engine
"""

import numpy as np

import concourse.bass as bass
import concourse.tile as tile
from concourse import bacc, mybir
from concourse.bass_utils import run_bass_kernel_spmd

AF = mybir.ActivationFunctionType
OP = mybir.AluOpType

PARENTS = np.array(
    [0, 0, 0, 0, 1, 2, 3, 4, 5, 6, 7, 8, 9, 9, 9, 12, 13, 14, 16, 17, 18, 19, 20, 21],
    dtype=np.int64,
)
J = 24
NCORES = 8
P = 128          # SBUF partitions
C = 98           # frame columns per (joint, quantity) block
FC = P * C       # frames per core = 12544
FTOT = NCORES * FC  # padded total = 100352
F = 100000
S = J * C        # comp-plane stride in the big tiles = 2352
UW = 5 * C       # u-tile block stride (max level width = 5 joints... use 490)

# (child0, n_children, parent0, parent_broadcast)
# compose groups: non-leaf joints only (leaves {10,11,15,22,23} never parent)
COMPOSE_GROUPS = [
    (1, 3, 0, True),
    (4, 3, 1, False),
    (7, 3, 4, False),
    (12, 3, 9, True),
    (16, 2, 13, False),
    (18, 2, 16, False),
    (20, 2, 18, False),
]
# position-update groups: every joint 1..23, children contiguous per group
P_GROUPS = [
    (1, 3, 0, True),
    (4, 3, 1, False),
    (7, 3, 4, False),
    (10, 3, 7, False),
    (13, 2, 9, True),
    (15, 3, 12, False),
    (18, 2, 16, False),
    (20, 2, 18, False),
    (22, 2, 20, False),
]

# const-tile column map
CT_T1 = 0     # (tz | tx | ty) planes of 24
CT_T2 = 72    # (ty | tz | tx)
CT_T0 = 144   # (tx | ty | tz)
CT_S1 = 216   # signs of x1-products  (-1, +1, -1, +1)
CT_S2 = 220   # signs of y1-products  (-1, +1, +1, -1)
CT_S3 = 224   # signs of z1-products  (-1, -1, +1, +1)
CT_HALFPI = 228
CT_C0 = 229   # canon[0] x,y,z
CT_W = 232

DT = mybir.dt.float32
NPDT = np.float32


def _ap(t: bass.AP, off: int, dims) -> bass.AP:
    """Build a custom AP on tile t: partition dim + given free [step, count] dims."""
    full = t
    pd = list(full.ap)[0]
    return bass.AP(
        tensor=full.tensor,
        offset=full.offset + off,
        ap=[list(pd)] + [list(d) for d in dims],
    )


def build_nc():
    nc = bacc.Bacc()
    poses_d = nc.dram_tensor("poses", [P, 3 * S], DT, kind="ExternalInput")
    consts_d = nc.dram_tensor("consts", [P, CT_W], DT, kind="ExternalInput")
    out_d = nc.dram_tensor("out", [P, 3 * S], DT, kind="ExternalOutput")

    with tile.TileContext(nc) as tc:
        with tc.tile_pool(name="persist", bufs=1) as perpool:
            CT = perpool.tile([P, CT_W], DT, name="CT")
            # Q: comp blocks [w | x | y | z | x' | y'], each S wide
            Q = perpool.tile([P, 6 * S], DT, name="Q")
            # P positions: comp planes [px | py | pz]
            PT = perpool.tile([P, 3 * S], DT, name="PT")
            # u scratch: blocks [ux | uy | uz | ux' | uy']
            U = perpool.tile([P, 5 * UW], DT, name="U")

            nc.sync.dma_start(CT[:, :], consts_d[:, :])

            # ---- Phase 1: local quaternions, 24 joints in 2 halves so the
            # ---- early tree levels can start composing while the ACT chain
            # ---- still runs on the second half of the joints.
            with (
                tc.tile_pool(name="ph1po", bufs=1) as ph1po,
                tc.tile_pool(name="ph1", bufs=3) as ph1,
            ):
                PO = ph1po.tile([P, 3 * S], DT, name="PO")
                # quats are only consumed for the 19 non-leaf joints; the
                # non-leaf set {0-9, 12-14, 16-21} is 3 contiguous runs, so
                # skip leaf columns entirely. The first chunk (joints 0-9)
                # covers levels 1-3 so composing starts while the ACT chain
                # still runs on the later joints.
                CHUNKS = [(0, 10 * C), (12 * C, 3 * C), (16 * C, 6 * C)]
                for o, H in CHUNKS:
                    for pl in range(3):
                        nc.sync.dma_start(
                            PO[:, pl * S + o : pl * S + o + H],
                            poses_d[:, pl * S + o : pl * S + o + H],
                        )
                tl = {}
                for h, (o, H) in enumerate(CHUNKS):
                    tl[h] = {
                        nm: ph1.tile([P, H], DT, name=f"{nm}{h}", tag=nm)
                        for nm in ("A", "B", "C2", "TH", "SH", "RT", "M")
                    }
                    A, B, C2 = tl[h]["A"], tl[h]["B"], tl[h]["C2"]
                    px = PO[:, o : o + H]
                    py = PO[:, S + o : S + o + H]
                    pz = PO[:, 2 * S + o : 2 * S + o + H]
                    # squares on DVE (idle during startup)
                    nc.vector.tensor_tensor(A[:, :], px, px, OP.mult)
                    nc.vector.tensor_tensor(B[:, :], py, py, OP.mult)
                    nc.vector.tensor_tensor(A[:, :], A[:, :], B[:, :], OP.add)
                    nc.vector.tensor_tensor(B[:, :], pz, pz, OP.mult)
                    # guarded theta^2: max(zz,eps)+(xx+yy) == max(theta2,eps)
                    # up to <=1e-12 absolute, fused in one STT
                    nc.vector.scalar_tensor_tensor(
                        C2[:, :], B[:, :], 1e-12, A[:, :], OP.max, OP.add
                    )
                # ACT ops batched BY TABLE SET across chunks -- per-function
                # interleaving costs an ~1.3us ACT_TABLE_LOAD per switch.
                # Ln+Exp share one set, so pairwise within that batch is free.
                for h in range(len(CHUNKS)):
                    nc.scalar.activation(
                        tl[h]["B"][:, :], tl[h]["C2"][:, :], AF.Ln
                    )
                    nc.scalar.activation(
                        tl[h]["RT"][:, :], tl[h]["B"][:, :], AF.Exp, scale=-0.5
                    )
                for h in range(len(CHUNKS)):
                    nc.scalar.activation(tl[h]["TH"][:, :], tl[h]["C2"][:, :], AF.Sqrt)
                for h, (o, H) in enumerate(CHUNKS):
                    nc.scalar.activation(
                        Q[:, o : o + H], tl[h]["TH"][:, :], AF.Sin,
                        bias=CT[:, CT_HALFPI : CT_HALFPI + 1], scale=0.5,
                    )
                    nc.scalar.activation(
                        tl[h]["SH"][:, :], tl[h]["TH"][:, :], AF.Sin, scale=0.5
                    )
                for h, (o, H) in enumerate(CHUNKS):
                    M, SH, RT = tl[h]["M"], tl[h]["SH"], tl[h]["RT"]
                    nc.vector.tensor_tensor(M[:, :], SH[:, :], RT[:, :], OP.mult)
                    nc.vector.tensor_tensor(
                        _ap(Q, S + o, [[S, 3], [1, H]]),
                        _ap(M, 0, [[0, 3], [1, H]]),
                        _ap(PO, o, [[S, 3], [1, H]]),
                        OP.mult,
                    )
                # dup x,y of root joint 0 into x',y'
                nc.vector.tensor_copy(
                    _ap(Q, 4 * S, [[S, 2], [1, C]]), _ap(Q, S, [[S, 2], [1, C]])
                )
                # seed P for joint 0 = canon[0]
                nc.vector.tensor_copy(
                    _ap(PT, 0, [[S, 3], [1, C]]), _ap(CT, CT_C0, [[1, 3], [0, C]])
                )

            # ---------------- Phase 2: tree levels --------------------------
            with tc.tile_pool(name="lvl", bufs=3) as tp:
                def compose_group(gi, c0, n, p0, bc):
                    nC = n * C
                    co, po = c0 * C, p0 * C
                    pstep = 0 if bc else 1
                    T0t = tp.tile([P, 4 * nC], DT, name=f"cp0_{gi}", tag="cp0")
                    T1t = tp.tile([P, 4 * nC], DT, name=f"cp1_{gi}", tag="cp1")
                    T2t = tp.tile([P, 4 * nC], DT, name=f"cp2_{gi}", tag="cp2")
                    T3t = tp.tile([P, 4 * nC], DT, name=f"cp3_{gi}", tag="cp3")
                    def par_b(blk):
                        if bc:
                            return _ap(Q, blk * S + po, [[0, 4], [0, n], [1, C]])
                        return _ap(Q, blk * S + po, [[0, 4], [1, nC]])

                    child4 = _ap(Q, 0 * S + co, [[S, 4], [1, nC]])
                    perm1 = _ap(Q, 1 * S + co, [[2 * S, 2], [-S, 2], [1, nC]])
                    perm2 = _ap(Q, 2 * S + co, [[-2 * S, 2], [S, 2], [1, nC]])
                    perm3 = _ap(Q, 3 * S + co, [[-2 * S, 2], [-S, 2], [1, nC]])
                    nc.vector.tensor_tensor(T0t[:, :], par_b(0), child4, OP.mult)
                    nc.vector.tensor_tensor(T1t[:, :], par_b(1), perm1, OP.mult)
                    nc.vector.tensor_tensor(T2t[:, :], par_b(2), perm2, OP.mult)
                    nc.vector.tensor_tensor(T3t[:, :], par_b(3), perm3, OP.mult)
                    # combine with signs folded into paired add/sub stages
                    # (A/B tiles hold blocks in (w,y,x,z) order):
                    #   qw = (P0w-P1w) - (P2w+P3w)    qx = (P0x+P1x) + (P2x-P3x)
                    #   qy = (P0y-P1y) + (P2y+P3y)    qz = (P0z+P1z) - (P2z-P3z)
                    At = tp.tile([P, 4 * nC], DT, name=f"cA_{gi}", tag="cA")
                    Bt = tp.tile([P, 4 * nC], DT, name=f"cB_{gi}", tag="cB")
                    wy_in = lambda t: _ap(t, 0, [[2 * nC, 2], [1, nC]])
                    xz_in = lambda t: _ap(t, nC, [[2 * nC, 2], [1, nC]])
                    nc.vector.tensor_tensor(
                        _ap(At, 0, [[1, 2 * nC]]), wy_in(T0t), wy_in(T1t), OP.subtract
                    )
                    nc.vector.tensor_tensor(
                        _ap(At, 2 * nC, [[1, 2 * nC]]), xz_in(T0t), xz_in(T1t), OP.add
                    )
                    nc.vector.tensor_tensor(
                        _ap(Bt, 0, [[1, 2 * nC]]), wy_in(T2t), wy_in(T3t), OP.add
                    )
                    nc.vector.tensor_tensor(
                        _ap(Bt, 2 * nC, [[1, 2 * nC]]), xz_in(T2t), xz_in(T3t),
                        OP.subtract,
                    )
                    # F: (w,z) = A-B -> Q blocks 0,3 ; (y,x) = A+B -> Q blocks 2,1
                    nc.vector.tensor_tensor(
                        _ap(Q, co, [[3 * S, 2], [1, nC]]),
                        _ap(At, 0, [[3 * nC, 2], [1, nC]]),
                        _ap(Bt, 0, [[3 * nC, 2], [1, nC]]),
                        OP.subtract,
                    )
                    nc.vector.tensor_tensor(
                        _ap(Q, 2 * S + co, [[-S, 2], [1, nC]]),
                        _ap(At, nC, [[nC, 2], [1, nC]]),
                        _ap(Bt, nC, [[nC, 2], [1, nC]]),
                        OP.add,
                    )
                    # dup new world x,y into x',y'
                    nc.vector.tensor_copy(
                        _ap(Q, 4 * S + co, [[S, 2], [1, nC]]),
                        _ap(Q, 1 * S + co, [[S, 2], [1, nC]]),
                    )

                def p_group(gi, c0, n, p0, bc):
                    nC = n * C
                    co, po = c0 * C, p0 * C

                    def prot(blk0):
                        if bc:
                            return _ap(Q, blk0 * S + po, [[S, 3], [0, n], [1, C]])
                        return _ap(Q, blk0 * S + po, [[S, 3], [1, nC]])

                    w1b = (
                        _ap(Q, po, [[0, 3], [0, n], [1, C]])
                        if bc
                        else _ap(Q, po, [[0, 3], [1, nC]])
                    )
                    rot1 = prot(2)  # (qy, qz, qx')
                    rot2 = prot(3)  # (qz, qx', qy')
                    # T1/T2 host-prescaled by 2 => u,v,h all carry the final 2x
                    T1b = _ap(CT, CT_T1 + c0, [[24, 3], [1, n], [0, C]])
                    T2b = _ap(CT, CT_T2 + c0, [[24, 3], [1, n], [0, C]])
                    T0b = _ap(CT, CT_T0 + c0, [[24, 3], [1, n], [0, C]])
                    Ma = tp.tile([P, 3 * nC], DT, name=f"pa_{gi}", tag="pa")
                    Mb = tp.tile([P, 3 * nC], DT, name=f"pb_{gi}", tag="pb")
                    PRE = tp.tile([P, 3 * nC], DT, name=f"pre_{gi}", tag="pre")
                    ppar = (
                        _ap(PT, po, [[S, 3], [0, n], [1, C]])
                        if bc
                        else _ap(PT, po, [[S, 3], [1, nC]])
                    )
                    # hoistable off the u/v critical chain
                    nc.vector.tensor_tensor(PRE[:, :], ppar, T0b, OP.add)
                    nc.vector.tensor_tensor(Ma[:, :], rot1, T1b, OP.mult)
                    nc.vector.tensor_tensor(Mb[:, :], rot2, T2b, OP.mult)
                    uout = _ap(U, 0, [[UW, 3], [1, nC]])
                    nc.vector.tensor_tensor(uout, Ma[:, :], Mb[:, :], OP.subtract)
                    nc.vector.tensor_copy(
                        _ap(U, 3 * UW, [[UW, 2], [1, nC]]),
                        _ap(U, 0, [[UW, 2], [1, nC]]),
                    )
                    urot1 = _ap(U, 1 * UW, [[UW, 3], [1, nC]])  # (uy, uz, ux')
                    urot2 = _ap(U, 2 * UW, [[UW, 3], [1, nC]])  # (uz, ux', uy')
                    nc.vector.tensor_tensor(Ma[:, :], rot1, urot2, OP.mult)
                    nc.vector.tensor_tensor(Mb[:, :], rot2, urot1, OP.mult)
                    nc.vector.tensor_tensor(Ma[:, :], Ma[:, :], Mb[:, :], OP.subtract)
                    u3 = _ap(U, 0, [[UW, 3], [1, nC]])
                    nc.vector.tensor_tensor(Mb[:, :], w1b, u3, OP.mult)
                    nc.vector.tensor_tensor(Ma[:, :], Mb[:, :], Ma[:, :], OP.add)
                    # p_child = g + (p_par + t)   (g pre-scaled via consts)
                    nc.vector.tensor_tensor(
                        _ap(PT, co, [[S, 3], [1, nC]]), Ma[:, :], PRE[:, :], OP.add
                    )
                    # stream out this group's columns while later levels run
                    nc.sync.dma_start(
                        _ap(out_d[:, :], co, [[S, 3], [1, nC]]),
                        _ap(PT, co, [[S, 3], [1, nC]]),
                    )

                # Emission order interleaves p-updates between compose levels
                # so the scheduler has DVE work to fill compose-chain waits:
                # H_k only needs level k-1 quats (already composed) + P chain.
                ORDER = [
                    ("p", 0), ("c", 0),          # H1 (needs only phase-1 j0) ; G1
                    ("p", 1), ("c", 1),          # H2 ; G2
                    ("p", 2), ("c", 2),          # H3 ; G3
                    ("p", 3), ("p", 4), ("c", 3),  # H4, H4b ; G4
                    ("p", 5), ("c", 4),          # H5 ; G5
                    ("p", 6), ("c", 5),          # H6 ; G6
                    ("p", 7), ("c", 6),          # H7 ; G7
                    ("p", 8),                    # H8
                ]
                for kind, gi in ORDER:
                    if kind == "c":
                        compose_group(gi, *COMPOSE_GROUPS[gi])
                    else:
                        p_group(gi, *P_GROUPS[gi])

            # joint-0 columns
            nc.sync.dma_start(
                _ap(out_d[:, :], 0, [[S, 3], [1, C]]), _ap(PT, 0, [[S, 3], [1, C]])
            )
    nc.compile()
    return nc


def _host_consts(canon: np.ndarray, parents: np.ndarray = None) -> np.ndarray:
    par = PARENTS if parents is None else np.asarray(parents).astype(np.int64)
    t = canon - canon[par]  # (24, 3)
    t2 = 2.0 * t  # fold the final "2x" of the rotation formula into u
    row = np.zeros(CT_W, dtype=NPDT)
    row[CT_T1 : CT_T1 + 24] = t2[:, 2]
    row[CT_T1 + 24 : CT_T1 + 48] = t2[:, 0]
    row[CT_T1 + 48 : CT_T1 + 72] = t2[:, 1]
    row[CT_T2 : CT_T2 + 24] = t2[:, 1]
    row[CT_T2 + 24 : CT_T2 + 48] = t2[:, 2]
    row[CT_T2 + 48 : CT_T2 + 72] = t2[:, 0]
    row[CT_T0 : CT_T0 + 24] = t[:, 0]
    row[CT_T0 + 24 : CT_T0 + 48] = t[:, 1]
    row[CT_T0 + 48 : CT_T0 + 72] = t[:, 2]
    row[CT_S1 : CT_S1 + 4] = [-1.0, 1.0, -1.0, 1.0]
    row[CT_S2 : CT_S2 + 4] = [-1.0, 1.0, 1.0, -1.0]
    row[CT_S3 : CT_S3 + 4] = [-1.0, -1.0, 1.0, 1.0]
    row[CT_HALFPI] = np.pi / 2
    row[CT_C0 : CT_C0 + 3] = canon[0]
    return np.broadcast_to(row, (P, CT_W)).copy()


def _shard_poses(poses: np.ndarray) -> np.ndarray:
    pad = np.broadcast_to(poses[-1:], (FTOT - F, J, 3))
    pp = np.concatenate([poses.astype(NPDT), pad.astype(NPDT)], axis=0)
    x = pp.reshape(NCORES, P, C, J, 3).transpose(0, 1, 4, 3, 2)  # core,p,comp,j,c
    return np.ascontiguousarray(x).reshape(NCORES, P, 3 * S)


def _unshard_out(outs: np.ndarray) -> np.ndarray:
    y = outs.reshape(NCORES, P, 3, J, C).transpose(0, 1, 4, 3, 2)  # core,p,c,j,comp
    return np.ascontiguousarray(y).reshape(FTOT, J, 3)[:F]


_NC_CACHE = {}


def kernel(poses, canon_kpts, parents):
    poses = np.asarray(poses)
    canon = np.asarray(canon_kpts, dtype=NPDT)
    if "nc" not in _NC_CACHE:
        _NC_CACHE["nc"] = build_nc()
    nc = _NC_CACHE["nc"]
    shards = _shard_poses(poses)
    consts = _host_consts(canon, parents)
    in_maps = [{"poses": shards[i], "consts": consts} for i in range(NCORES)]
    res = run_bass_kernel_spmd(nc, in_maps, list(range(NCORES))).results
    outs = np.stack([r["out"] for r in res])
    return _unshard_out(outs).astype(np.float32)
